# revision 64
# baseline (speedup 1.0000x reference)
"""Trainium2 Bass kernel for FlashMultiHeadAttention (B=2, L=2048, D=1024, H=16, Dh=64).

Sharding: 8 cores = 2 (batch) x 4 (head groups of 4 heads).
Per core (batch b, head group hg, 4 heads):
  - Q^T/K^T projections ([256, L], head dims on partitions, 8 k-tiles; bq/bk
    folded into the ACT-engine PSUM->SBUF evacuation as per-partition biases).
    RoPE applied by the DVE entirely in SBUF bf16 (4x DVE rate) on the ACT
    copies.  U (9 k-tiles, action-gate + biases via augmented rows) and V
    (9 k-tiles, ones-row bias) projected in natural [L, 256] layout; sigmoid
    via ACT tanh; gating mul fused into one DVE op per chunk.
  - Scores computed transposed (S^T[k, q]); exp producers alternate by key
    block between the ACT engine (exp with fused scale+bias+cast) and the DVE
    (Schraudolph int16 bitcast-to-bf16 approximation) so softmax never paces
    the PE.  P^T feeds PV directly; PV carries a denominator ones-column.
  - Normalization: fast-approx reciprocal + DRAM-broadcast of 1/r; the final
    per-head scale runs on GpSimd (SBUF only), off every critical engine.
  - Row-sliced output projection -> partial outT [1024, 2048] bf16, with the
    two 128-row halves of the contraction accumulated n2-major so the last
    head's normalize overlaps the first half of the out-projection.
Host sums the 4 head-group partials per batch and adds bo.

One PSUM pool with two 2-bank tags ("qp"/"up", bufs=2 each = 8 banks) is
shared by every phase so there are no pool-release barriers.  Input DMAs are
spread over four rings (SP: xq+xv, Pool: xk+wk/wv/wo, PE: wq/wu, ACT: small
tables + first xq chunk tail) so no single ring serializes the head.
"""

import sys

if "/opt/trn_rl_repo" not in sys.path:
    sys.path.insert(0, "/opt/trn_rl_repo")

import numpy as np
import ml_dtypes

BF16 = ml_dtypes.bfloat16

B = 2
L = 2048
D = 1024
H = 16
DH = 64
NG = 256          # head dims per group (4 heads)
NCORES = 8
SCALE = DH ** -0.5

# Schraudolph exp approximation constants (bf16 bit layout via int16).
A16 = 128.0 / float(np.log(2.0))
SIGMA = 0.0579
B0 = 128.0 * (127.0 - SIGMA)
# exp producer split: head A (even head of each pair) runs on the DVE
# (Schraudolph) as two half-tiles so its st banks release early enough to
# never stall the next kt's score matmuls (a recurring micro-gap makes the
# PE HAM clock-gate collapse to 1.2 GHz); head B runs on the ACT engine as
# one full tile (its scores come second in the kt, so the later release is
# safe).  L2 error adds in quadrature, so concentrating the Schraudolph
# approximation on one head costs the same global error as spreading it.


def build_bass(with_mask: bool):
    """Build the single-core SPMD Bass program (same program on all 8 cores)."""
    import concourse.mybir as mybir
    from concourse import bacc
    from concourse.tile import TileContext

    f32 = mybir.dt.float32
    bf16 = mybir.dt.bfloat16
    i16 = mybir.dt.int16
    EXP = mybir.ActivationFunctionType.Exp
    TANH = mybir.ActivationFunctionType.Tanh
    IDENT = mybir.ActivationFunctionType.Identity
    COPY = mybir.ActivationFunctionType.Copy
    RECIP = mybir.ActivationFunctionType.Reciprocal
    MULT = mybir.AluOpType.mult
    ADD = mybir.AluOpType.add

    nc = bacc.Bacc(None, target_bir_lowering=False)

    # x inputs packed host-side as [128, chunk, d, 512] and weights as
    # [128, d, NG] so every DMA descriptor moves 4-8KB per partition row
    # (the DMA engines are descriptor-rate-bound near 1KB/row).
    xq = nc.dram_tensor("xq", [128, 4 * 8 * 512], bf16, kind="ExternalInput")
    xq8 = nc.dram_tensor("xq8", [17, L], bf16, kind="ExternalInput")
    xk = nc.dram_tensor("xk", [128, 4 * 8 * 512], bf16, kind="ExternalInput")
    xv = nc.dram_tensor("xv", [128, 4 * 8 * 512], bf16, kind="ExternalInput")
    wq = nc.dram_tensor("wq", [128, 8 * NG], bf16, kind="ExternalInput")
    wu = nc.dram_tensor("wu", [128, 9 * NG], bf16, kind="ExternalInput")
    wk = nc.dram_tensor("wk", [128, 8 * NG], bf16, kind="ExternalInput")
    wv = nc.dram_tensor("wv", [128, 9 * NG], bf16, kind="ExternalInput")
    wo = nc.dram_tensor("wo", [NG, D], bf16, kind="ExternalInput")
    cb = nc.dram_tensor("cb", [128, 64], f32, kind="ExternalInput")
    bqk = nc.dram_tensor("bqk", [128, 4], f32, kind="ExternalInput")
    cs = nc.dram_tensor("cs", [128, L], bf16, kind="ExternalInput")
    sn = nc.dram_tensor("sn", [128, L], bf16, kind="ExternalInput")
    mk = None
    if with_mask:
        mk = nc.dram_tensor("mk", [L, L], f32, kind="ExternalInput")
    outT = nc.dram_tensor("outT", [D, L], bf16, kind="ExternalOutput")

    with TileContext(nc) as tc:
        with tc.tile_pool(name="persist", bufs=1) as persist, \
             tc.tile_pool(name="xbuf", bufs=1) as xbuf, \
             tc.tile_pool(name="ps", bufs=1, space="PSUM") as ps, \
             tc.tile_pool(name="ev", bufs=2) as ev, \
             tc.tile_pool(name="ptp", bufs=6) as ptpool, \
             tc.tile_pool(name="otp", bufs=2) as otpool, \
             tc.tile_pool(name="drm", bufs=2, space="DRAM") as drm, \
             tc.tile_pool(name="mkp", bufs=4) as mkpool:
            qT = [persist.tile([128, L], bf16, name=f"qT{n}") for n in range(2)]
            kT = [persist.tile([128, L], bf16, name=f"kT{n}") for n in range(2)]
            vg = persist.tile([128, 16 * 260], bf16, name="vg")
            vg4 = vg.rearrange("p (t h e) -> p t h e", h=4, e=65)
            sig = [persist.tile([128, 1024], bf16, name=f"sig{c}") for c in range(4)]
            oT = [persist.tile([128, L], bf16, name=f"oT{n}") for n in range(2)]
            csS = persist.tile([128, L], bf16, name="csS")
            snS = persist.tile([128, L], bf16, name="snS")
            cbS = persist.tile([128, 64], f32, name="cbS")
            cbA = persist.tile([128, 64], f32, name="cbA")
            bqkS = persist.tile([128, 4], f32, name="bqkS")
            woS = [persist.tile([128, D], bf16, name=f"woS{n2}") for n2 in range(2)]
            wqS = persist.tile([128, 8 * NG], bf16, name="wqS")
            wuS = persist.tile([128, 9 * NG], bf16, name="wuS")
            wkS = persist.tile([128, 8 * NG], bf16, name="wkS")
            wvS = persist.tile([128, 9 * NG], bf16, name="wvS")
            ib = persist.tile([128, L], f32, name="ib")
            # rg/rinv: rows qc*32 hold 1/r; head A in cols 0:512, B in 512:1024
            rg = persist.tile([128, 1024], f32, name="rg")
            rinv = persist.tile([128, 1024], f32, name="rinv")

            wqS3 = wqS.rearrange("p (c n) -> p c n", n=NG)
            wuS3 = wuS.rearrange("p (c n) -> p c n", n=NG)
            wkS3 = wkS.rearrange("p (c n) -> p c n", n=NG)
            wvS3 = wvS.rearrange("p (c n) -> p c n", n=NG)

            # per-chunk xq tiles (xv reuses the same bytes chunk-by-chunk);
            # the d=8 padding rows never move: U's 17 action/bias rows load
            # once into xq8S, V's ones row is a memset constant.
            xqC = [xbuf.tile([128, 8 * 512], bf16, tag=f"xq{c}",
                             name=f"xqC{c}") for c in range(4)]
            xqC3 = [t.rearrange("p (d q) -> p d q", q=512) for t in xqC]
            xkS = xbuf.tile([128, 4 * 8 * 512], bf16, tag="xB", name="xkS")
            xkS4 = xkS.rearrange("p (c d q) -> p c d q", d=8, q=512)
            xq8S = persist.tile([128, L], bf16, name="xq8S")
            vone = persist.tile([128, 128], bf16, name="vone")

            # ---- input DMAs: block copies over three rings ----
            nc.gpsimd.memset(xq8S, 0.0)
            nc.gpsimd.memset(vone, 0.0)
            nc.gpsimd.memset(vone[0:1, :], 1.0)
            xvC = [xbuf.tile([128, 8 * 512], bf16, tag=f"xq{c}",
                             name=f"xvC{c}") for c in range(4)]
            xvC3 = [t.rearrange("p (d q) -> p d q", q=512) for t in xvC]

            # ACT ring: wq, xq c0 tail, wu, xq c3, table first halves.
            nc.scalar.dma_start(out=wqS[:, 0:1024], in_=wq[:, 0:1024])
            nc.scalar.dma_start(out=xqC[0][:, 2048:3072], in_=xq[:, 2048:3072])
            nc.scalar.dma_start(out=wqS[:, 1024:2048], in_=wq[:, 1024:2048])
            nc.scalar.dma_start(out=xqC[0][:, 3072:4096], in_=xq[:, 3072:4096])
            for o in range(0, 2304, 1152):
                nc.scalar.dma_start(out=wuS[:, o:o + 1152],
                                    in_=wu[:, o:o + 1152])
            nc.scalar.dma_start(out=xq8S[0:17, :], in_=xq8[:, :])
            nc.scalar.dma_start(out=csS[:, 0:1024], in_=cs[:, 0:1024])
            nc.scalar.dma_start(out=snS[:, 0:1024], in_=sn[:, 0:1024])
            # SP ring: xq c0 head, c1, c2, xk c3, then all xv chunks
            # (split into 2KB-row pieces so the DGE pipelines them).
            def split_dma(ring, dst, dsrc, lo, hi, step=1024):
                for o in range(lo, hi, step):
                    ring.dma_start(out=dst[:, o - lo:o - lo + step],
                                   in_=dsrc[:, o:o + step])
            split_dma(nc.sync, xqC[0], xq, 0, 2048)
            split_dma(nc.sync, xqC[1], xq, 4096, 8192)
            split_dma(nc.sync, xvC[0], xv, 0, 4096)
            # Pool ring: K/V weights, xk c0-c2, tables, wo.
            nc.gpsimd.dma_start(out=bqkS, in_=bqk[:, :])
            nc.gpsimd.dma_start(out=csS[:, 1024:2048], in_=cs[:, 1024:2048])
            split_dma(nc.gpsimd, xqC[2], xq, 8192, 12288)
            split_dma(nc.gpsimd, wkS, wk, 0, 2048)
            split_dma(nc.gpsimd, xqC[3], xq, 12288, 16384)
            for c in range(4):
                split_dma(nc.gpsimd, xkS[:, c * 4096:(c + 1) * 4096], xk,
                          c * 4096, (c + 1) * 4096)
            nc.gpsimd.dma_start(out=snS[:, 1024:2048], in_=sn[:, 1024:2048])
            split_dma(nc.gpsimd, wvS, wv, 0, 2304, 1152)
            nc.gpsimd.dma_start(out=cbS, in_=cb[:, :])
            for c in (2, 3):
                split_dma(nc.gpsimd, xvC[c], xv, c * 4096, (c + 1) * 4096)
            for n2 in range(2):
                nc.gpsimd.dma_start(out=woS[n2],
                                    in_=wo[n2 * 128:(n2 + 1) * 128, :])

            # device-side preamble computations
            nc.vector.tensor_scalar(cbA, cbS, A16, B0, MULT, ADD)
            nc.vector.memset(vg4[:, :, :, 64:65], 1.0)
            nc.gpsimd.memset(rg, 1.0)

            # head dims are stored pair-interleaved (partner of p is p^1), so
            # rotate_half is a swap of adjacent partitions within quadrants.
            SWAP_MASK = [i ^ 1 for i in range(32)]

            def rope_sbuf(raw, dest, s):
                """dest[:, s] = raw*cos + rotate_half(raw)*signed_sin (bf16 SBUF)."""
                for n in range(2):
                    src = raw[:, n * 512:(n + 1) * 512]
                    tcx = ev.tile([128, 512], bf16, tag="tc", bufs=2, name="tcx")
                    rot = ev.tile([128, 512], bf16, tag="tr", bufs=2, name="rot")
                    nc.vector.tensor_mul(tcx, src, csS[:, s])
                    nc.vector.stream_shuffle(rot, src, SWAP_MASK)
                    nc.vector.tensor_mul(rot, rot, snS[:, s])
                    nc.vector.tensor_add(dest[n][:, s], tcx, rot)

            # ---- QU phase ----
            for c in range(4):
                s = slice(c * 512, (c + 1) * 512)
                qps = ps.tile([128, 1024], f32, tag="up", bufs=2, name="qps")
                for d in range(8):
                    xt = xqC3[c][:, d, :]
                    for n in range(2):
                        nc.tensor.matmul(qps[:, n * 512:(n + 1) * 512],
                                         lhsT=wqS3[:, d, n * 128:(n + 1) * 128],
                                         rhs=xt, start=(d == 0), stop=(d == 7))
                qraw = ev.tile([128, 1024], bf16, tag="qraw", bufs=2, name="qraw")
                for n in range(2):
                    nc.scalar.activation(out=qraw[:, n * 512:(n + 1) * 512],
                                         in_=qps[:, n * 512:(n + 1) * 512],
                                         func=IDENT, bias=bqkS[:, n:n + 1])
                rope_sbuf(qraw, qT, s)
                ups = ps.tile([128, 1024], f32, tag="up", bufs=2, name="ups")
                for i in range(4):
                    for d in range(9):
                        if d < 8:
                            lhsT = xqC3[c][:, d, i * 128:(i + 1) * 128]
                        else:
                            lhsT = xq8S[:, c * 512 + i * 128:
                                        c * 512 + (i + 1) * 128]
                        nc.tensor.matmul(ups[:, i * 256:(i + 1) * 256],
                                         lhsT=lhsT,
                                         rhs=wuS3[:, d, :],
                                         start=(d == 0), stop=(d == 8))
                eu = ev.tile([128, 1024], bf16, tag="eu", bufs=2, name="eu")
                nc.scalar.activation(out=eu, in_=ups, func=TANH, scale=0.5)
                # sigmoid(u) = 0.5*tanh(0.5*u) + 0.5
                nc.vector.tensor_scalar(sig[c], eu, 0.5, 0.5, MULT, ADD)

            # late xv chunks on the ACT ring (emitted after the QU loop so
            # their WAR kicks sit behind the QU evacs on this queue)
            for c in (1, 3):
                split_dma(nc.scalar, xvC[c], xv, c * 4096, (c + 1) * 4096)

            # ---- K phase (V chunks are deferred into the first attention
            # pass so their xv loads can trickle in past the DMA-bound
            # projection window) ----
            for c in range(4):
                s = slice(c * 512, (c + 1) * 512)
                kps = ps.tile([128, 1024], f32, tag="up", bufs=2, name="kps")
                for d in range(8):
                    xt = xkS4[:, c, d, :]
                    for n in range(2):
                        nc.tensor.matmul(kps[:, n * 512:(n + 1) * 512],
                                         lhsT=wkS3[:, d, n * 128:(n + 1) * 128],
                                         rhs=xt, start=(d == 0), stop=(d == 7))
                kraw = ev.tile([128, 1024], bf16, tag="qraw", bufs=2, name="kraw")
                for n in range(2):
                    nc.scalar.activation(out=kraw[:, n * 512:(n + 1) * 512],
                                         in_=kps[:, n * 512:(n + 1) * 512],
                                         func=IDENT, bias=bqkS[:, 2 + n:3 + n])
                rope_sbuf(kraw, kT, s)

            def v_chunk(c, gate_eng):
                vps = ps.tile([128, 1024], f32, tag="qb", bufs=1,
                              name=f"vps{c}")
                for i in range(4):
                    for d in range(9):
                        if d < 8:
                            lhsT = xvC3[c][:, d, i * 128:(i + 1) * 128]
                        else:
                            lhsT = vone
                        nc.tensor.matmul(vps[:, i * 256:(i + 1) * 256],
                                         lhsT=lhsT,
                                         rhs=wvS3[:, d, :],
                                         start=(d == 0), stop=(d == 8))
                vraw = ev.tile([128, 1024], bf16, tag="eu", bufs=2, name="vraw")
                nc.scalar.activation(out=vraw, in_=vps, func=COPY)
                gate_eng.tensor_mul(
                    vg4[:, c * 4:(c + 1) * 4, :, 0:64],
                    vraw.rearrange("p (i h e) -> p i h e", h=4, e=64),
                    sig[c].rearrange("p (i h e) -> p i h e", h=4, e=64))

            # ---- Attention (merged head-pair passes) ----
            # Pass = (pair n, hq half), order (1,0),(1,1),(0,0),(0,1) so
            # oT[1] completes first for the n2=1-major out-projection.
            # Within a pass the two heads' score matmuls run CONCURRENTLY as
            # 64x128 row tiles ((0,0)/(64,0)): kT/qT already stack the pair
            # at partitions 0-63/64-127, so the pair's scores cost one MM
            # span instead of two.  PV stays M=65 (ones column = softmax
            # denominator) serial per head in full 128x128 mode.  Per kt the
            # lagged PV runs first, then the score pair, so each st buffer
            # has a full PV phase of exp runway before its bank is reused
            # (st is double-buffered).  exp alternates ACT/DVE per (head,kt)
            # so each kt costs max(ACT, DVE), not their sum.  The normalize
            # chain runs per (pass) on its q-half so the out-projection's
            # early qc blocks unblock long before the last pass ends.
            def epilogue2(h, n, hq, pvtH):
                hoff = (h % 2) * 512
                r0 = (h % 2) * 64
                for s2 in range(2):
                    qc = hq * 2 + s2
                    csl = slice(s2 * 512, s2 * 512 + 512)
                    rdst = rg[qc * 32:qc * 32 + 1, hoff:hoff + 512]
                    nc.vector.tensor_copy(out=rdst, in_=pvtH[64:65, csl])
                for s2 in range(2):
                    csl = slice(s2 * 512, s2 * 512 + 512)
                    q0 = hq * 1024 + s2 * 512
                    dst = oT[n][r0:r0 + 64, q0:q0 + 512]
                    nc.scalar.activation(out=dst, in_=pvtH[0:64, csl],
                                         func=COPY)

            def half_chain(n, hq, last):
                # normalize BOTH heads' q-half: reciprocal -> DRAM-broadcast
                # of 1/r -> per-head scale (gpsimd off the critical engines,
                # vector for the final chain that gates the out-projection).
                nc.vector.reciprocal_approx_fast(out=rinv, in_=rg)
                r3 = rinv.rearrange("(a b) f -> a b f", b=32)
                bsl = slice(hq * 1024, hq * 1024 + 1024)
                for h2 in range(2):
                    r0 = h2 * 64
                    drv = drm.tile([2, 512], f32, tag="drv", bufs=4,
                                   name=f"drv{n}_{hq}_{h2}")
                    nc.sync.dma_start(
                        out=drv,
                        in_=r3[2 * hq:2 * hq + 2, 0, h2 * 512:h2 * 512 + 512])
                    ring = nc.gpsimd if h2 == 0 else nc.sync
                    ring.dma_start(out=ib[r0:r0 + 64, bsl],
                                   in_=drv.flatten()[:].partition_broadcast(64))
                    eng = nc.vector if last else nc.gpsimd
                    eng.tensor_mul(oT[n][r0:r0 + 64, bsl],
                                   oT[n][r0:r0 + 64, bsl], ib[r0:r0 + 64, bsl])

            pass_done = {}

            def flush_one():
                pt, kt, fh, fn, fhq, fpvt = pending.pop(0)
                for s2 in range(2):
                    nc.tensor.matmul(
                        fpvt[0:65, s2 * 512:(s2 + 1) * 512],
                        lhsT=vg[:, kt * 260 + fh * 65:kt * 260 + fh * 65 + 65],
                        rhs=pt[:, s2 * 512:(s2 + 1) * 512],
                        start=(kt == 0), stop=(kt == 15))
                if kt == 15:
                    epilogue2(fh, fn, fhq, fpvt)
                    k2 = (fn, fhq)
                    pass_done[k2] = pass_done.get(k2, 0) + 1
                    if pass_done[k2] == 2:
                        half_chain(fn, fhq, last=(fn == 0 and fhq == 1))

            # V-chunk insertion points inside the first pass: early chunks
            # right away (PV consumes vg kt-progressively), later chunks
            # spaced so their xv DMAs have time to land.
            V_AT = {}
            for vc in range(4):
                v_chunk(vc, nc.vector)
            pending = []
            for n, hq in ((1, 0), (1, 1), (0, 0), (0, 1)):
                pvt = [ps.tile([128, 1024], f32, tag="up", bufs=2,
                               name=f"pvt{n}_{hq}_{i}") for i in range(2)]
                for kt in range(16):
                    if n == 1 and hq == 0 and kt in V_AT:
                        vc, veng = V_AT[kt]
                        v_chunk(vc, veng)
                    while len(pending) > 4:
                        flush_one()
                    # head B (ACT exp) scores go FIRST so its full-tile exp
                    # starts as early as possible in the period; head A (DVE
                    # Schraudolph) uses two single-bank tiles so each half
                    # releases as soon as its own exp half has read it.
                    stB = ps.tile([128, 1024], f32, tag="qb", bufs=1, name="stB")
                    for s2 in range(2):
                        q0 = hq * 1024 + s2 * 512
                        nc.tensor.matmul(
                            stB[:, s2 * 512:(s2 + 1) * 512],
                            lhsT=kT[n][64:128, kt * 128:(kt + 1) * 128],
                            rhs=qT[n][64:128, q0:q0 + 512],
                            start=True, stop=True)
                    colB = kt * 4 + 2 * n + 1
                    ptB = ptpool.tile([128, 1024], bf16, tag="pt", name="ptB")
                    if with_mask:
                        mt_ = mkpool.tile([128, 1024], f32, tag="mt", name="mt")
                        nc.sync.dma_start(
                            out=mt_,
                            in_=mk[kt * 128:(kt + 1) * 128,
                                   hq * 1024:(hq + 1) * 1024])
                        nc.vector.tensor_add(stB, stB, mt_)
                    nc.scalar.activation(out=ptB, in_=stB, func=EXP,
                                         scale=SCALE,
                                         bias=cbS[:, colB:colB + 1])
                    stA = [ps.tile([128, 512], f32, tag=f"qa{s2}", bufs=1,
                                   name=f"stA{s2}") for s2 in range(2)]
                    for s2 in range(2):
                        q0 = hq * 1024 + s2 * 512
                        nc.tensor.matmul(
                            stA[s2],
                            lhsT=kT[n][0:64, kt * 128:(kt + 1) * 128],
                            rhs=qT[n][0:64, q0:q0 + 512],
                            start=True, stop=True)
                    colA = kt * 4 + 2 * n
                    ptA = ptpool.tile([128, 1024], bf16, tag="pt", name="ptA")
                    if with_mask:
                        mtA = mkpool.tile([128, 1024], f32, tag="mt", name="mtA")
                        nc.sync.dma_start(
                            out=mtA,
                            in_=mk[kt * 128:(kt + 1) * 128,
                                   hq * 1024:(hq + 1) * 1024])
                        for s2 in range(2):
                            csl = slice(s2 * 512, s2 * 512 + 512)
                            nc.vector.tensor_add(stA[s2], stA[s2], mtA[:, csl])
                            nc.scalar.activation(
                                out=ptA[:, csl], in_=stA[s2], func=EXP,
                                scale=SCALE, bias=cbS[:, colA:colA + 1])
                    else:
                        for s2 in range(2):
                            csl = slice(s2 * 512, s2 * 512 + 512)
                            nc.vector.tensor_scalar(
                                ptA[:, csl].bitcast(i16), stA[s2],
                                A16 * SCALE, cbA[:, colA:colA + 1],
                                MULT, ADD)
                    pending.append((ptB, kt, 2 * n + 1, n, hq, pvt[1]))
                    pending.append((ptA, kt, 2 * n, n, hq, pvt[0]))
                if n == 0 and hq == 1:
                    while pending:
                        flush_one()

            # keep the PE p-state hot across the final normalize half-chain
            ogw = ps.tile([128, 1024], f32, tag="qb", bufs=1, name="ogwarm")
            ogw = ogw[:, 0:512]
            for j in range(4):
                nc.tensor.matmul(ogw, lhsT=woS[0][:, 0:128],
                                 rhs=woS[0][:, 0:512],
                                 start=(j == 0), stop=(j == 3))

            # ---- Out-projection (n2-major accumulation) ----
            # two sweeps: all mt's qc0/1 columns first (their normalize
            # chains finished passes ago -> real work covers the final
            # chain's latency), then the qc2/3 columns.
            for half in range(2):
                for mt_i in range(8):
                    if mt_i % 2 == 0:
                        og0 = [ps.tile([128, 512], f32, tag=f"qa{j}", bufs=1,
                                       name=f"og{mt_i}_{half}_{j}")
                               for j in range(2)]
                    else:
                        ogu = ps.tile([128, 1024], f32, tag="up", bufs=2,
                                      name=f"og{mt_i}_{half}")
                        og0 = [ogu[:, 0:512], ogu[:, 512:1024]]
                    for n2 in (1, 0):
                        for j in range(2):
                            qc = half * 2 + j
                            nc.tensor.matmul(
                                og0[j],
                                lhsT=woS[n2][:, mt_i * 128:(mt_i + 1) * 128],
                                rhs=oT[n2][:, qc * 512:(qc + 1) * 512],
                                start=(n2 == 1), stop=(n2 == 0))
                    ot = otpool.tile([128, 1024], bf16, tag=f"ot{half}",
                                     bufs=2, name=f"ot{mt_i}_{half}")
                    for j in range(2):
                        csl = slice(j * 512, (j + 1) * 512)
                        if j == 0:
                            nc.scalar.activation(out=ot[:, csl], in_=og0[j],
                                                 func=COPY)
                        else:
                            nc.vector.tensor_copy(out=ot[:, csl], in_=og0[j])
                    ring = (nc.sync, nc.gpsimd, nc.scalar)[mt_i % 3]
                    ring.dma_start(
                        out=outT[mt_i * 128:(mt_i + 1) * 128,
                                 half * 1024:(half + 1) * 1024],
                        in_=ot)

    nc.finalize()
    return nc


def prep_inputs(query, key, value, attn_mask, action_ids, time_deltas,
                Wq, bq, Wk, bk, Wv, bv, Wu, bu, Wo, bo,
                action_emb, Wap, bap, td_emb, td_gate):
    """Host-side sharding: build the 8 per-core input maps."""
    query = np.asarray(query, np.float32)
    key = np.asarray(key, np.float32)
    value = np.asarray(value, np.float32)
    attn_mask = np.asarray(attn_mask)
    action_ids = np.asarray(action_ids)
    time_deltas = np.asarray(time_deltas)
    Wq, bq = np.asarray(Wq, np.float32), np.asarray(bq, np.float32)
    Wk, bk = np.asarray(Wk, np.float32), np.asarray(bk, np.float32)
    Wv, bv = np.asarray(Wv, np.float32), np.asarray(bv, np.float32)
    Wu, bu = np.asarray(Wu, np.float32), np.asarray(bu, np.float32)
    Wap, bap = np.asarray(Wap, np.float32), np.asarray(bap, np.float32)

    sig_gate = 1.0 / (1.0 + np.exp(-np.float64(td_gate)))
    with_mask = not bool(attn_mask.all())

    def chunk_pack(xT):
        """[1024 feat, L] -> [128, (4 chunks, 8 d, 512)] packed bf16."""
        x = np.ascontiguousarray(xT).astype(BF16)                   # [1024, L]
        x = x.reshape(8, 128, 4, 512).transpose(1, 2, 0, 3)
        return np.ascontiguousarray(x.reshape(128, 4 * 8 * 512))

    xq_b, xq8_b, xk_b, xv_b, cb_b, mk_b = [], [], [], [], [], []
    for b in range(B):
        ae = np.asarray(action_emb, np.float32)[action_ids[b]]      # [L, 16]
        xq_b.append(chunk_pack(query[b].T))
        x8 = np.empty((17, L), BF16)
        x8[0:16] = ae.T.astype(BF16)
        x8[16] = BF16(1.0)
        xq8_b.append(x8)
        xk_b.append(chunk_pack(key[b].T))
        xv_b.append(chunk_pack(value[b].T))
        tdc = np.clip(time_deltas[b].astype(np.int64), 0, td_emb.shape[0] - 1)
        cb_b.append((sig_gate * np.asarray(td_emb, np.float32)[tdc]).astype(np.float32))
        if with_mask:
            m = np.where(attn_mask[b], np.float32(0.0), np.float32(-1e9))
            mk_b.append(np.ascontiguousarray(m.T))                  # [k, q]

    wu_a = np.zeros((9 * 128, D), np.float32)
    wu_a[:D] = Wu
    wu_a[D:D + 16] = Wap
    wu_a[D + 16] = bu + bap
    wv_a = np.zeros((9 * 128, D), np.float32)
    wv_a[:D] = Wv
    wv_a[D] = bv

    def w_pack(w):
        """[nd*128, 256] weight -> [128, (nd, 256)] row-contiguous bf16."""
        nd = w.shape[0] // 128
        wp = w.astype(BF16).reshape(nd, 128, NG).transpose(1, 0, 2)
        return np.ascontiguousarray(wp.reshape(128, nd * NG))

    # RoPE tables in [dh, pos] orientation, duplicated for the 2-head packing.
    # Head dims are stored pair-interleaved (perm64) so the rotate_half
    # partner of partition p is p^1 (a 32-lane stream_shuffle pair swap); the
    # sin table carries the rotate_half sign.
    inv_freq = 1.0 / (10000.0 ** (np.arange(0, DH, 2, dtype=np.float64) / DH))
    pos = np.arange(L, dtype=np.float64)
    freqs = pos[None, :] * inv_freq[:, None]            # [32, L]
    cos_t = np.repeat(np.cos(freqs), 2, axis=0)[:DH]    # [64, L]
    sin_t = np.repeat(np.sin(freqs), 2, axis=0)[:DH]
    ss_t = sin_t.copy()
    ss_t[0:32] = -ss_t[0:32]
    perm64 = np.empty(DH, np.int64)
    perm64[0::2] = np.arange(32)
    perm64[1::2] = np.arange(32) + 32
    gperm = np.concatenate([h * DH + perm64 for h in range(4)])     # [256]
    cos_p, ss_p = cos_t[perm64], ss_t[perm64]
    cs_t = np.ascontiguousarray(np.concatenate([cos_p, cos_p], 0)).astype(BF16)
    sn_t = np.ascontiguousarray(np.concatenate([ss_p, ss_p], 0)).astype(BF16)

    in_maps = []
    for c in range(NCORES):
        b, hg = c // 4, c % 4
        csl = slice(hg * NG, (hg + 1) * NG)
        cbc = cb_b[b][:, hg * 4:(hg + 1) * 4]                       # [L, 4]
        cbc = cbc.reshape(16, 128, 4).transpose(1, 0, 2).reshape(128, 64)
        bq_g, bk_g = bq[csl][gperm], bk[csl][gperm]
        bqk_t = np.zeros((128, 4), np.float32)
        bqk_t[:, 0] = bq_g[0:128]
        bqk_t[:, 1] = bq_g[128:256]
        bqk_t[:, 2] = bk_g[0:128]
        bqk_t[:, 3] = bk_g[128:256]
        m = {
            "xq": xq_b[b], "xq8": xq8_b[b], "xk": xk_b[b], "xv": xv_b[b],
            "wq": w_pack(np.ascontiguousarray(Wq[:, csl][:, gperm])),
            "wu": w_pack(wu_a[:, csl]),
            "wk": w_pack(np.ascontiguousarray(Wk[:, csl][:, gperm])),
            "wv": w_pack(wv_a[:, csl]),
            "wo": np.asarray(Wo, np.float32)[csl, :].astype(BF16),
            "cb": np.ascontiguousarray(cbc, np.float32),
            "bqk": bqk_t,
            "cs": cs_t, "sn": sn_t,
        }
        if with_mask:
            m["mk"] = mk_b[b]
        in_maps.append(m)
    return in_maps, with_mask


def gather_output(results, bo):
    """Sum head-group partials per batch, transpose, add bo."""
    out = np.empty((B, L, D), np.float32)
    for b in range(B):
        acc = results[b * 4]["outT"].astype(np.float32)
        for g in range(1, 4):
            acc = acc + results[b * 4 + g]["outT"].astype(np.float32)
        out[b] = acc.T + np.asarray(bo, np.float32)
    return out


def kernel(**inputs):
    from concourse.bass_utils import run_bass_kernel_spmd

    in_maps, with_mask = prep_inputs(**inputs)
    nc = build_bass(with_mask)
    res = run_bass_kernel_spmd(nc, in_maps, core_ids=list(range(NCORES)))
    return gather_output(res.results, inputs["bo"])

